# revision 8
# baseline (speedup 1.0000x reference)
"""Trainium2 Bass kernel for the DiscMaker mkaarma/controller scan.

Math per step t (per batch element b):
    ns    = tanh(x_t @ Wx[j] + kstate @ Wh[j])          j=0..2   [B,3,S]
    enc   = tanh(x_t @ We + kstate @ Ue)                         [B,E]
    cst   = tanh([enc, err] @ Wi + cst @ Whc)                    [B,H]
    out   = cst @ Wo                                             [B,4]
    gate  = softmax(out[:, :3] @ Wd + bd) ; theta = sigmoid(out[:, 3])
    gate  = gate*theta + gate_prev*(1-theta)
    kstate= sum_j gate[:,j] * ns[:,j,:] ; pred = kstate[:,-1] ; err = pred - y_t

Device design (per core, batch shard b=32, feature-on-partition):
  - kstate is never materialized: we carry G[s,(j,b)] = gate[j,b]*ns[s,j,b] and
    every consumer of kstate contracts G with 3 accumulating matmuls.
  - err is never materialized: err = sum_j G[127,j,:] - y, and its only use is
    the rank-1 row of Wi, so it becomes 4 K=1 matmuls (3 G127 slices + (-y)).
  - theta via the identity sigmoid(z) = (1+tanh(z/2))/2 so the whole loop uses
    one ACT table set {tanh, exp}. The gate head is folded: Wfold = [Wo[:, :3]@Wd,
    0.5*Wo[:,3]], bias row appended via a K=1 matmul.
  - gate algebra on DVE in batch-on-partition [32,*]:
        q = e * (1/Z) ;  d = q - gate_prev ; s = q + gate_prev   (gf2 = 2*gate)
        gf2_new = th2*d + s      (== 2 * gate_new)
    then a 32x32 DVE transpose + a PE broadcast with 0.5-ones gives
    gateB[128,(j,b)], and one DVE multiply forms G = ns * gateB.
  - preds come from G_all[127,:] which is DMA'd out once; host sums over j.
"""

import os
import sys
import tempfile

import numpy as np

sys.path.insert(0, "/opt/trn_rl_repo")

import concourse.bass as bass  # noqa: E402
import concourse.tile as tile  # noqa: E402
from concourse import bacc, mybir  # noqa: E402

F16 = mybir.dt.float16
F32 = mybir.dt.float32
AF = mybir.ActivationFunctionType
ALU = mybir.AluOpType

B, T, D_IN, S, E, H, NOUT = 256, 512, 64, 128, 128, 256, 4
NCORES = 8
BC = B // NCORES  # 32 batch per core


def build_program(T_steps=T):
    nc = bacc.Bacc(
        "TRN2", target_bir_lowering=False, debug=False, enable_asserts=False
    )
    Tn = T_steps

    def din(name, shape, dt=F16):
        return nc.dram_tensor(name, shape, dt, kind="ExternalInput").ap()

    xw = din("xw", [64, Tn * BC])          # x[b,t,d] -> [d, t*32+b]
    negy = din("negy", [1, Tn * BC])       # -y[b,t]  -> [1, t*32+b]
    gf20 = din("gf20", [BC, 32])           # 2*gate0 in cols 0:3, zeros elsewhere
    whcat = din("whcat", [S, 3 * S])       # Wh[j][s_in, s_out] stacked on free
    wxcat = din("wxcat", [D_IN, 3 * S])
    ue = din("ue", [S, E])
    we = din("we", [D_IN, E])
    wit = din("wit", [E, H])               # Wi[0:128]
    wib = din("wib", [1, H])               # Wi[128]
    whc0 = din("whc0", [S, H])             # Whc[0:128]
    whc1 = din("whc1", [S, H])             # Whc[128:256]
    wfold = din("wfold", [S, 8])           # [F[0:128] | F[128:256]], F=[Wo@Wd_ext]
    bdext = din("bdext", [1, 4])           # [bd, 0]
    halfI = din("halfI", [BC, BC])         # 0.5 * I_32
    ones132 = din("ones132", [1, BC])      # 1.0

    g127o = nc.dram_tensor("g127", [1, Tn * 96], F16, kind="ExternalOutput").ap()

    with TileCtx(nc) as tc:
        import contextlib
        stk = contextlib.ExitStack()
        persist = stk.enter_context(tc.tile_pool(name="persist", bufs=1))

        def _tile(_tc, shape, dtype, name):
            return persist.tile(shape, dtype, name=name, tag=name)

        # ---- persistent SBUF ----
        s_xw = _tile(tc, [64, Tn * BC], F16, name="s_xw")
        s_negy = _tile(tc, [1, Tn * BC], F16, name="s_negy")
        s_whcat = _tile(tc, [S, 3 * S], F16, name="s_whcat")
        s_wxcat = _tile(tc, [D_IN, 3 * S], F16, name="s_wxcat")
        s_ue = _tile(tc, [S, E], F16, name="s_ue")
        s_we = _tile(tc, [D_IN, E], F16, name="s_we")
        s_wit = _tile(tc, [E, H], F16, name="s_wit")
        s_wib = _tile(tc, [1, H], F16, name="s_wib")
        s_whc0 = _tile(tc, [S, H], F16, name="s_whc0")
        s_whc1 = _tile(tc, [S, H], F16, name="s_whc1")
        s_wfold = _tile(tc, [S, 8], F16, name="s_wfold")
        s_bdext = _tile(tc, [1, 4], F16, name="s_bdext")
        s_halfI = _tile(tc, [BC, BC], F16, name="s_halfI")
        s_ones132 = _tile(tc, [1, BC], F16, name="s_ones132")
        s_gall = _tile(tc, [S, Tn * 96], F16, name="s_gall")
        s_cst = _tile(tc, [S, 2 * BC], F16, name="s_cst")  # cstate halves on free
        gf2a = _tile(tc, [BC, 32], F16, name="gf2a")
        gf2b = _tile(tc, [BC, 32], F16, name="gf2b")

        for dst, src in [
            (s_xw, xw), (s_negy, negy), (s_whcat, whcat), (s_wxcat, wxcat),
            (s_ue, ue), (s_we, we), (s_wit, wit), (s_wib, wib),
            (s_whc0, whc0), (s_whc1, whc1), (s_wfold, wfold),
            (s_bdext, bdext), (s_halfI, halfI), (s_ones132, ones132),
            (gf2a, gf20),
        ]:
            nc.sync.dma_start(out=dst[:], in_=src)
        nc.vector.memset(gf2b[:], 0.0)

        # ---- pools ----
        pEnc = stk.enter_context(tc.tile_pool(name="pEnc", bufs=1, space="PSUM"))
        pNs = stk.enter_context(tc.tile_pool(name="pNs", bufs=2, space="PSUM"))
        pB = stk.enter_context(tc.tile_pool(name="pB", bufs=1, space="PSUM"))
        pG = stk.enter_context(tc.tile_pool(name="pG", bufs=1, space="PSUM"))
        pGB = stk.enter_context(tc.tile_pool(name="pGB", bufs=1, space="PSUM"))
        wk = stk.enter_context(tc.tile_pool(name="wk", bufs=3))

        ts = bass.ts
        gf2 = [gf2a, gf2b]

        for t in range(Tn):
            xt = s_xw[:, ts(t, BC)]
            gprev = None if t == 0 else s_gall[:, ts(t - 1, 96)]

            # --- stage A: enc + 3 candidate branches, pre-activation in PSUM
            a_enc = pEnc.tile([S, BC], F32, tag="a_enc")
            nc.tensor.matmul(a_enc[:], s_we[:], xt, start=True, stop=(t == 0))
            if t > 0:
                for j in range(3):
                    nc.tensor.matmul(
                        a_enc[:], s_ue[:],
                        gprev[:, ts(j, BC)], start=False, stop=(j == 2),
                    )
            a_ns = pNs.tile([S, 96], F32, tag="a_ns")
            for k in range(3):
                sl = a_ns[:, ts(k, BC)]
                nc.tensor.matmul(
                    sl, s_wxcat[:, ts(k, S)], xt, start=True, stop=(t == 0)
                )
                if t > 0:
                    for j in range(3):
                        nc.tensor.matmul(
                            sl, s_whcat[:, ts(k, S)],
                            gprev[:, ts(j, BC)], start=False, stop=(j == 2),
                        )

            # --- tanh(enc) first (on the critical chain)
            A_enc = wk.tile([S, BC], F16, tag="A_enc")
            nc.scalar.activation(A_enc[:], a_enc[:], AF.Tanh)

            # --- controller pre-activation [128, 64] (two halves on free)
            b_ = pB.tile([S, 2 * BC], F32, tag="b_")
            for h in range(2):
                sl = b_[:, ts(h, BC)]
                first = True
                if t > 0:
                    nc.tensor.matmul(sl, s_whc0[:, ts(h, S)], s_cst[:, 0:BC],
                                     start=True, stop=False)
                    nc.tensor.matmul(sl, s_whc1[:, ts(h, S)], s_cst[:, BC:2 * BC],
                                     start=False, stop=False)
                    wib_l = s_wib[:, ts(h, S)]
                    for j in range(3):
                        nc.tensor.matmul(
                            sl, wib_l,
                            s_gall[0:1, (t - 1) * 96 + j * BC:(t - 1) * 96 + (j + 1) * BC],
                            start=False, stop=False)
                    nc.tensor.matmul(sl, wib_l, s_negy[:, ts(t - 1, BC)],
                                     start=False, stop=False)
                    first = False
                nc.tensor.matmul(sl, s_wit[:, ts(h, S)], A_enc[:],
                                 start=first, stop=True)

            # --- tanh -> cstate (fp16, feeds matmuls)
            nc.scalar.activation(s_cst[:], b_[:], AF.Tanh)

            # --- gate head: gate_pre [32, 4] = cst @ Wfold + bd_ext
            g = pG.tile([BC, 4], F32, tag="g")
            nc.tensor.matmul(g[:], s_cst[:, 0:BC], s_wfold[:, 0:4],
                             start=True, stop=False)
            nc.tensor.matmul(g[:], s_cst[:, BC:2 * BC], s_wfold[:, 4:8],
                             start=False, stop=False)
            nc.tensor.matmul(g[:], s_ones132[:], s_bdext[:],
                             start=False, stop=True)

            # --- exp + row-sum, th2 = tanh(o3/2)
            e = wk.tile([BC, 3], F32, tag="e")
            z = wk.tile([BC, 1], F32, tag="z")
            nc.scalar.activation(e[:], g[:, 0:3], AF.Exp, accum_out=z[:])
            th2 = wk.tile([BC, 1], F32, tag="th2")
            nc.scalar.activation(th2[:], g[:, 3:4], AF.Tanh)
            # ns tanh emitted here: off the critical path until the G multiply
            A_ns = wk.tile([S, 96], F16, tag="A_ns")
            nc.scalar.activation(A_ns[:], a_ns[:], AF.Tanh)

            # --- gate algebra on DVE
            r0 = wk.tile([BC, 1], F32, tag="r0")
            nc.vector.reciprocal(r0[:], z[:])
            q = wk.tile([BC, 3], F32, tag="q")
            nc.vector.tensor_scalar_mul(q[:], e[:], r0[:])
            gp = gf2[t % 2]
            gn = gf2[(t + 1) % 2]
            dd = wk.tile([BC, 3], F32, tag="dd")
            nc.vector.scalar_tensor_tensor(dd[:], gp[:, 0:3], -0.5, q[:],
                                           ALU.mult, ALU.add)
            ss = wk.tile([BC, 3], F32, tag="ss")
            nc.vector.scalar_tensor_tensor(ss[:], gp[:, 0:3], 0.5, q[:],
                                           ALU.mult, ALU.add)
            nc.vector.scalar_tensor_tensor(gn[:, 0:3], dd[:], th2[:], ss[:],
                                           ALU.mult, ALU.add)

            # --- gate broadcast: out[s, b] = sum_k gn[k, j] * 0.5*I[k, b]
            #     = 0.5 * gf2[b, j]  (stride-0 free lhsT broadcast)
            gb = pGB.tile([S, 96], F32, tag="gb")
            for j in range(3):
                nc.tensor.matmul(gb[:, ts(j, BC)],
                                 gn[:, j:j + 1].broadcast_to([BC, S]),
                                 s_halfI[:], start=True, stop=True)
            nc.vector.tensor_mul(s_gall[:, ts(t, 96)], A_ns[:], gb[:])

        nc.sync.dma_start(out=g127o, in_=s_gall[0:1, :])
        stk.close()
    nc.finalize()
    return nc




def TileCtx(nc):
    return tile.TileContext(nc)


# ---------------- host side ----------------

def _pack_inputs(x, y, Wx, Wh, We, Ue, Wi, Whc, Wo, Wd, bd, gate0, Tn=T):
    """Build the 8 per-core input dicts."""
    f16 = np.float16
    F = np.concatenate(
        [Wo[:, :3] @ Wd, 0.5 * Wo[:, 3:4]], axis=1
    ).astype(np.float32)  # [256, 4]
    # permute the S dim so the prediction feature (s=127) sits on partition 0
    # (matmul operands must have base partition 0/32/64)
    perm = np.arange(S)
    perm[[0, S - 1]] = [S - 1, 0]
    Whp = [Wh[j][perm][:, perm] for j in range(3)]
    Wxp = [Wx[j][:, perm] for j in range(3)]
    shared = {
        "whcat": np.concatenate(Whp, axis=1).astype(f16),
        "wxcat": np.concatenate(Wxp, axis=1).astype(f16),
        "ue": Ue[perm, :].astype(f16),
        "we": We.astype(f16),
        "wit": Wi[0:E].astype(f16),
        "wib": Wi[E:E + 1].astype(f16),
        "whc0": Whc[0:S].astype(f16),
        "whc1": Whc[S:2 * S].astype(f16),
        "wfold": np.concatenate([F[0:S], F[S:2 * S]], axis=1).astype(f16),
        "bdext": np.concatenate([bd, [0.0]]).reshape(1, 4).astype(f16),
        "halfI": (0.5 * np.eye(BC)).astype(f16),
        "ones132": np.ones((1, BC), f16),
    }
    in_maps = []
    for c in range(NCORES):
        bs = slice(c * BC, (c + 1) * BC)
        xs = x[bs, :Tn]                      # [32, T, 64]
        ys = y[bs, :Tn]                      # [32, T]
        g0 = gate0[bs]                       # [32, 3]
        gf20 = np.zeros((BC, 32), np.float32)
        gf20[:, 0:3] = 2.0 * g0
        m = dict(shared)
        m["xw"] = np.ascontiguousarray(
            xs.transpose(2, 1, 0).reshape(64, Tn * BC)
        ).astype(f16)
        m["negy"] = np.ascontiguousarray(
            (-ys.T).reshape(1, Tn * BC)
        ).astype(f16)
        m["gf20"] = gf20.astype(f16)
        in_maps.append(m)
    return in_maps


_PROG_CACHE = {}
LAST_RESULT = {}


def kernel(x, y, Wx, Wh, We, Ue, Wi, Whc, Wo, Wd, bd, gate0):
    from concourse.bass_utils import run_bass_kernel_spmd

    args = [np.asarray(a, dtype=np.float32) for a in
            (x, y, Wx, Wh, We, Ue, Wi, Whc, Wo, Wd, bd, gate0)]
    in_maps = _pack_inputs(*args)
    if "prog" not in _PROG_CACHE:
        _PROG_CACHE["prog"] = build_program(T)
    nc = _PROG_CACHE["prog"]
    trace = bool(int(os.environ.get("TRN_KERNEL_TRACE", "0")))
    res = run_bass_kernel_spmd(
        nc, in_maps, core_ids=list(range(NCORES)), trace=trace
    )
    LAST_RESULT["exec_time_ns"] = res.exec_time_ns
    LAST_RESULT["res"] = res
    preds = np.zeros((B, T), np.float32)
    for c in range(NCORES):
        g127 = res.results[c]["g127"].reshape(T, 3, BC).astype(np.float32)
        preds[c * BC:(c + 1) * BC] = g127.sum(axis=1).T
    return preds
